# revision 15
# baseline (speedup 1.0000x reference)
"""GCN encoder (2-layer GCNConv: relu(GCN(x)) -> mu, logstd) on 8 TRN2 NeuronCores.

Strategy (per sharding hint): shard nodes/edges by destination across the 8
cores. Per core:
  Phase A: full h = (x @ W1) * dis  computed redundantly (row-major, DRAM).
  Phase B: edge aggregation for own dst-shard: indexed dma_gather of
           hs[src] rows (f32, 512B descriptors) + one-hot Sel matmul
           segment-reduction on the PE into PSUM (agg^T, channel-major).
  Phase C: h2s = dis * relu(dis*agg1 + b1): own shard, transposed to
           row-major via TensorE.
  Phase D: AllGather h2s shards -> full gather source for layer 2.
  Phase E: layer-2 aggregation (same edge structure, new gather source).
  Phase F: mu^T = W_mu^T @ (dis*agg2) + b_mu, same for logstd; DMA out.
Self-loops are appended to the edge list on the host (weight dis[n]^2 ==
standard edge norm), so no separate self-loop term is needed.
Host does index-only preprocessing: degree counting, sorting edges by
(dst-shard, src-half, dst-group), padding to a core-uniform tile structure.
"""

import os
import sys

sys.path.insert(0, "/opt/trn_rl_repo")

import numpy as np

N_NODES = 50000
N_EDGES = 1_600_000
IN_CH, HID_CH, OUT_CH = 256, 128, 64
N_CORES = 8
NPC = 6250              # real nodes per core
NPC_PAD = 6272          # padded (x64) nodes per core
W_SEL = 64              # Sel width = nodes per PSUM group
GROUPS = NPC_PAD // W_SEL  # 98
TILE_E = 128            # edges per matmul tile
CALL_TILES = 32         # tiles per dma_gather call (4096 idxs)
ROWS = N_NODES + 48     # x rows padded to x128 (50048)
# L1 gather-source layout: nodes 0..25087 at rows 0.., 64 zero rows,
# nodes 25088..50047 at rows 25152.., 64 zero rows. Total 50176.
SPLIT1 = 25088
HS_ROWS = 50176
# L2 source: AllGather output, shard c at rows c*6272, cols 6250.. zeroed.
SPLIT2_NODE = 25000     # nodes < 25000 live in rows < 25088


def _ceil(a, b):
    return -(-a // b)


def preprocess(edge_index):
    """Index-only host preprocessing. Returns per-core device arrays and the
    compile-time tile structure (core-uniform)."""
    src = edge_index[0].astype(np.int64)
    dst = edge_index[1].astype(np.int64)
    deg = 1.0 + np.bincount(dst, minlength=N_NODES).astype(np.float64)
    dis = (1.0 / np.sqrt(deg)).astype(np.float32)

    sl = np.arange(N_NODES, dtype=np.int64)
    src_a = np.concatenate([src, sl])
    dst_a = np.concatenate([dst, sl])

    core = dst_a // NPC
    dstloc = dst_a % NPC
    grp = dstloc // W_SEL
    rel = (dstloc - grp * W_SEL).astype(np.float32)

    # L1 gather rows (node-major with zero-row gaps); L2 rows (shard-major).
    row1 = src_a + 64 * (src_a >= SPLIT1)
    half1 = (src_a >= SPLIT1).astype(np.int64)
    loc1 = row1 - half1 * (SPLIT1 + 64)
    row2 = (src_a // NPC) * NPC_PAD + (src_a % NPC)
    half2 = (src_a >= SPLIT2_NODE).astype(np.int64)
    loc2 = row2 - half2 * SPLIT1
    DUMMY1 = (SPLIT1, 24960)       # zero rows, local idx per half
    DUMMY2 = (NPC, NPC)            # shard-0 pad row 6250 / shard-4 pad row

    # Counts per (core, layer-half, group) -> uniform K across cores.
    def structure(half):
        key = (core * 2 + half) * GROUPS + grp
        cnt = np.bincount(key, minlength=N_CORES * 2 * GROUPS).reshape(
            N_CORES, 2, GROUPS)
        K = _ceil(cnt.max(axis=0), TILE_E)  # [2, GROUPS]
        return cnt, K

    cnt1, K1 = structure(half1)
    cnt2, K2 = structure(half2)
    # Round each half's tile total up to CALL_TILES by bumping last group.
    for K in (K1, K2):
        for h in range(2):
            tot = int(K[h].sum())
            K[h, GROUPS - 1] += _ceil(tot, CALL_TILES) * CALL_TILES - tot

    def build_layer(halfl, locl, K, dummy):
        T = [int(K[h].sum()) for h in range(2)]
        idx_all = np.empty((N_CORES, 128, (T[0] + T[1]) * 8), np.int16)
        rel_all = np.zeros((N_CORES, 128, T[0] + T[1]), np.float32)
        # group start offsets (in edges) within each half stream
        off = [np.concatenate([[0], np.cumsum(K[h]) * TILE_E]) for h in (0, 1)]
        order = np.lexsort((grp, halfl, core))
        s_core, s_half, s_grp = core[order], halfl[order], grp[order]
        s_loc, s_rel = locl[order], rel[order]
        # boundaries per (core, half, group)
        key = (s_core * 2 + s_half) * GROUPS + s_grp
        starts = np.searchsorted(key, np.arange(N_CORES * 2 * GROUPS + 1))
        for c in range(N_CORES):
            flat_idx = np.empty((T[0] + T[1]) * TILE_E, np.int64)
            flat_rel = np.zeros((T[0] + T[1]) * TILE_E, np.float32)
            for h in range(2):
                base = 0 if h == 0 else T[0] * TILE_E
                flat_idx[base:base + T[h] * TILE_E] = dummy[h]
                for g in range(GROUPS):
                    k = (c * 2 + h) * GROUPS + g
                    a, b = starts[k], starts[k + 1]
                    n = b - a
                    o = base + int(off[h][g])
                    flat_idx[o:o + n] = s_loc[a:b]
                    flat_rel[o:o + n] = s_rel[a:b]
            ia = flat_idx.reshape(-1, 16).T.astype(np.int16)  # [16, T*8]
            idx_all[c] = np.tile(ia, (8, 1))
            rel_all[c] = flat_rel.reshape(-1, 128).T
        return T, idx_all, rel_all

    T1, idx1, rel1 = build_layer(half1, loc1, K1, DUMMY1)
    T2, idx2, rel2 = build_layer(half2, loc2, K2, DUMMY2)
    return dict(dis=dis, K1=K1, K2=K2, T1=T1, T2=T2,
                idx1=idx1, rel1=rel1, idx2=idx2, rel2=rel2)


def build_program(K1, K2, T1, T2, phases="ABCDEF"):
    import concourse.bass as bass
    import concourse.bacc as bacc
    import concourse.mybir as mybir
    import concourse.tile as tile

    DT = mybir.dt
    F32 = DT.float32
    nc = bacc.Bacc(None, target_bir_lowering=False, debug=False)

    xT = nc.declare_dram_parameter("xT", [IN_CH, ROWS], F32, isOutput=False)
    w1 = nc.declare_dram_parameter("w1", [IN_CH, HID_CH], F32, isOutput=False)
    b1 = nc.declare_dram_parameter("b1", [HID_CH, 1], F32, isOutput=False)
    wmu = nc.declare_dram_parameter("wmu", [HID_CH, OUT_CH], F32, isOutput=False)
    bmu = nc.declare_dram_parameter("bmu", [OUT_CH, 1], F32, isOutput=False)
    wls = nc.declare_dram_parameter("wls", [HID_CH, OUT_CH], F32, isOutput=False)
    bls = nc.declare_dram_parameter("bls", [OUT_CH, 1], F32, isOutput=False)
    disp = nc.declare_dram_parameter("disp", [128, ROWS // 128], F32, isOutput=False)
    disb = nc.declare_dram_parameter("disb", [128, NPC_PAD], F32, isOutput=False)
    iot = nc.declare_dram_parameter("iot", [128, W_SEL], F32, isOutput=False)
    ident = nc.declare_dram_parameter("ident", [128, 128], F32, isOutput=False)
    idx1e = nc.declare_dram_parameter("idx1", [128, (T1[0] + T1[1]) * 8], DT.int16, isOutput=False)
    rel1e = nc.declare_dram_parameter("rel1", [128, T1[0] + T1[1]], F32, isOutput=False)
    idx2e = nc.declare_dram_parameter("idx2", [128, (T2[0] + T2[1]) * 8], DT.int16, isOutput=False)
    rel2e = nc.declare_dram_parameter("rel2", [128, T2[0] + T2[1]], F32, isOutput=False)
    mu_t = nc.declare_dram_parameter("mu_t", [OUT_CH, NPC_PAD], F32, isOutput=True)
    ls_t = nc.declare_dram_parameter("ls_t", [OUT_CH, NPC_PAD], F32, isOutput=True)

    hs1 = nc.dram_tensor("hs1", [HS_ROWS, HID_CH], F32)

    with tile.TileContext(nc, num_cores=N_CORES) as tc:
        with (
            tc.tile_pool(name="const", bufs=1) as cpool,
            tc.tile_pool(name="xa", bufs=3) as xpool,
            tc.tile_pool(name="pa", bufs=2, space="PSUM") as papool,
            tc.tile_pool(name="ha", bufs=3) as hapool,
            tc.tile_pool(name="gb", bufs=2) as gpool,
            tc.tile_pool(name="ib", bufs=3) as ipool,
            tc.tile_pool(name="sel", bufs=6) as spool,
            tc.tile_pool(name="pg", bufs=2, space="PSUM") as pgpool,
            tc.tile_pool(name="agg", bufs=1) as apool,
            tc.tile_pool(name="dram", bufs=1, space="DRAM") as dpool,
            tc.tile_pool(name="pt", bufs=2, space="PSUM") as ptpool,
            tc.tile_pool(name="po", bufs=2, space="PSUM") as popool,
            tc.tile_pool(name="outp", bufs=1) as opool,
        ):
            # ---- constants ----
            w1_sb = cpool.tile([128, 2, HID_CH], F32)  # two k-tiles
            nc.sync.dma_start(w1_sb[:, 0, :], w1[0:128, :])
            nc.sync.dma_start(w1_sb[:, 1, :], w1[128:256, :])
            b1_sb = cpool.tile([HID_CH, 1], F32)
            nc.sync.dma_start(b1_sb[:], b1[:])
            wmu_sb = cpool.tile([HID_CH, OUT_CH], F32)
            nc.sync.dma_start(wmu_sb[:], wmu[:])
            bmu_sb = cpool.tile([OUT_CH, 1], F32)
            nc.sync.dma_start(bmu_sb[:], bmu[:])
            wls_sb = cpool.tile([HID_CH, OUT_CH], F32)
            nc.sync.dma_start(wls_sb[:], wls[:])
            bls_sb = cpool.tile([OUT_CH, 1], F32)
            nc.sync.dma_start(bls_sb[:], bls[:])
            disp_sb = cpool.tile([128, ROWS // 128], F32)
            nc.sync.dma_start(disp_sb[:], disp[:])
            disb_sb = cpool.tile([128, NPC_PAD], F32)
            nc.sync.dma_start(disb_sb[:], disb[:])
            io_sb = cpool.tile([128, W_SEL], F32)
            nc.sync.dma_start(io_sb[:], iot[:])
            id_sb = cpool.tile([128, 128], F32)
            nc.sync.dma_start(id_sb[:], ident[:])
            zero_sb = cpool.tile([128, HID_CH], F32)
            nc.vector.memset(zero_sb[:], 0.0)
            # zero rows in hs1
            nc.sync.dma_start(hs1[SPLIT1:SPLIT1 + 64, :], zero_sb[0:64, :])
            nc.sync.dma_start(hs1[HS_ROWS - 64:HS_ROWS, :], zero_sb[0:64, :])

            # ---- Phase A: hs1 = (x @ W1) * dis, row-major, all rows ----
            for rt in range(ROWS // 128 if "A" in phases else 2):
                xk0 = xpool.tile([128, 128], F32, tag="xk0")
                xk1 = xpool.tile([128, 128], F32, tag="xk1")
                nc.sync.dma_start(xk0[:], xT[0:128, rt * 128:(rt + 1) * 128])
                nc.sync.dma_start(xk1[:], xT[128:256, rt * 128:(rt + 1) * 128])
                ps = papool.tile([128, HID_CH], F32, tag="psA")
                nc.tensor.matmul(ps[:], xk0[:], w1_sb[:, 0, :], start=True, stop=False)
                nc.tensor.matmul(ps[:], xk1[:], w1_sb[:, 1, :], start=False, stop=True)
                hrow = hapool.tile([128, HID_CH], F32, tag="hrow")
                nc.scalar.activation(
                    hrow[:], ps[:], mybir.ActivationFunctionType.Identity,
                    scale=disp_sb[:, rt:rt + 1])
                lo = rt * 128 + (64 if rt * 128 >= SPLIT1 else 0)
                nc.sync.dma_start(hs1[lo:lo + 128, :], hrow[:])

            # ---- aggregation phase (shared for both layers) ----
            def aggregate(K, T, idx_ext, rel_ext, srcA, srcB, aggT):
                rel_sb = cpool.tile([128, T[0] + T[1]], F32,
                                    tag=f"rel{idx_ext.name}")
                nc.sync.dma_start(rel_sb[:], rel_ext[:])
                for h, src_ap in ((0, srcA), (1, srcB)):
                    tbase = 0 if h == 0 else T[0]
                    n_calls = T[h] // CALL_TILES
                    # tile t (within half) -> (group, k) schedule
                    sched = []
                    for g in range(GROUPS):
                        for k in range(int(K[h][g])):
                            sched.append((g, k, k == 0, k == int(K[h][g]) - 1))
                    assert len(sched) == T[h]
                    ps = None
                    for call in range(n_calls):
                        ixt = ipool.tile([128, CALL_TILES * 8], DT.int16, tag="ixt")
                        c0 = (tbase + call * CALL_TILES) * 8
                        nc.sync.dma_start(ixt[:], idx_ext[:, c0:c0 + CALL_TILES * 8])
                        G = gpool.tile([128, CALL_TILES, HID_CH], F32, tag="G")
                        nc.gpsimd.dma_gather(
                            G[:], src_ap, ixt[:],
                            CALL_TILES * TILE_E, CALL_TILES * TILE_E, HID_CH,
                            single_packet=False)
                        for tl in range(CALL_TILES):
                            t = call * CALL_TILES + tl
                            g, k, first, last = sched[t]
                            sel = spool.tile([128, W_SEL], F32, tag="sel")
                            nc.vector.tensor_scalar(
                                out=sel[:], in0=io_sb[:],
                                scalar1=rel_sb[:, tbase + t:tbase + t + 1],
                                scalar2=None, op0=mybir.AluOpType.is_equal)
                            if first:
                                ps = pgpool.tile([128, W_SEL], F32, tag="psG")
                            nc.tensor.matmul(ps[:], G[:, tl, :], sel[:],
                                             start=first, stop=last)
                            if last:
                                sl = aggT[:, g * W_SEL:(g + 1) * W_SEL]
                                if h == 0:
                                    nc.vector.tensor_copy(sl, ps[:])
                                else:
                                    nc.vector.tensor_tensor(
                                        out=sl, in0=sl, in1=ps[:],
                                        op=mybir.AluOpType.add)
                    # groups never opened in half A (K=0): zero them
                    if h == 0:
                        for g in range(GROUPS):
                            if int(K[0][g]) == 0:
                                nc.vector.memset(
                                    aggT[:, g * W_SEL:(g + 1) * W_SEL], 0.0)

            # ---- Phase B: layer-1 aggregation ----
            agg1 = apool.tile([128, NPC_PAD], F32)
            if "B" in phases:
                aggregate(K1, T1, idx1e, rel1e,
                          hs1[0:SPLIT1 + 64, :], hs1[SPLIT1 + 64:HS_ROWS, :],
                          agg1)
            else:
                nc.vector.memset(agg1[:], 0.0)

            # ---- Phase C: h2s = dis * relu(dis*agg1 + b1), transpose ----
            h2s = agg1  # in-place to save SBUF
            nc.vector.tensor_tensor(out=h2s[:], in0=agg1[:], in1=disb_sb[:],
                                    op=mybir.AluOpType.mult)
            nc.scalar.activation(h2s[:], h2s[:],
                                 mybir.ActivationFunctionType.Relu,
                                 bias=b1_sb[:])
            nc.vector.tensor_tensor(out=h2s[:], in0=h2s[:], in1=disb_sb[:],
                                    op=mybir.AluOpType.mult)
            ag_in = dpool.tile([NPC_PAD, HID_CH], F32)
            ag_out = dpool.tile([N_CORES * NPC_PAD, HID_CH], F32,
                                addr_space="Shared")
            for j in range(NPC_PAD // 128 if "C" in phases else 2):
                pt = ptpool.tile([128, 128], F32, tag="pt")
                nc.tensor.transpose(pt[:], h2s[:, j * 128:(j + 1) * 128], id_sb[:])
                rowt = hapool.tile([128, 128], F32, tag="rowt")
                nc.vector.tensor_copy(rowt[:], pt[:])
                nc.sync.dma_start(ag_in[j * 128:(j + 1) * 128, :], rowt[:])

            # ---- Phase D: AllGather ----
            if "D" in phases:
                nc.gpsimd.collective_compute(
                    "AllGather", mybir.AluOpType.bypass,
                    replica_groups=[list(range(N_CORES))],
                    ins=[ag_in.opt()], outs=[ag_out.opt()])
            else:
                nc.sync.dma_start(ag_out[0:NPC_PAD, :], ag_in[:])

            # ---- Phase E: layer-2 aggregation ----
            agg2 = apool.tile([128, NPC_PAD], F32)
            if "E" in phases:
                aggregate(K2, T2, idx2e, rel2e,
                          ag_out[0:SPLIT1, :],
                          ag_out[SPLIT1:N_CORES * NPC_PAD, :], agg2)
            else:
                nc.vector.memset(agg2[:], 0.0)

            # ---- Phase F: outputs ----
            nc.vector.tensor_tensor(out=agg2[:], in0=agg2[:], in1=disb_sb[:],
                                    op=mybir.AluOpType.mult)
            for (w_sb, b_sb, out_ext) in ((wmu_sb, bmu_sb, mu_t),
                                          (wls_sb, bls_sb, ls_t)):
                ot = opool.tile([OUT_CH, NPC_PAD], F32,
                                tag=f"ot{out_ext.name}")
                for j in range(_ceil(NPC_PAD, 512)):
                    n0 = j * 512
                    n1 = min(NPC_PAD, n0 + 512)
                    po = popool.tile([OUT_CH, 512], F32, tag="po")
                    nc.tensor.matmul(po[:, 0:n1 - n0], w_sb[:],
                                     agg2[:, n0:n1], start=True, stop=True)
                    nc.vector.tensor_scalar(
                        out=ot[:, n0:n1], in0=po[:, 0:n1 - n0],
                        scalar1=b_sb[:], scalar2=None,
                        op0=mybir.AluOpType.add)
                nc.sync.dma_start(out_ext[:], ot[:])
    nc.compile()
    return nc


def make_in_maps(x, W1, b1, W_mu, b_mu, W_ls, b_ls, pp):
    dis = pp["dis"]
    dis_pad = np.zeros(ROWS, np.float32)
    dis_pad[:N_NODES] = dis
    xT_full = np.zeros((IN_CH, ROWS), np.float32)
    xT_full[:, :N_NODES] = np.asarray(x, np.float32).T
    disp = dis_pad.reshape(ROWS // 128, 128).T.copy()
    iot = np.tile(np.arange(W_SEL, dtype=np.float32)[None, :], (128, 1))
    ident = np.eye(128, dtype=np.float32)
    in_maps = []
    for c in range(N_CORES):
        disb = np.zeros((128, NPC_PAD), np.float32)
        disb[:, :NPC] = np.tile(dis[c * NPC:(c + 1) * NPC][None, :], (128, 1))
        in_maps.append({
            "xT": xT_full, "w1": np.asarray(W1, np.float32),
            "b1": np.asarray(b1, np.float32).reshape(HID_CH, 1),
            "wmu": np.asarray(W_mu, np.float32),
            "bmu": np.asarray(b_mu, np.float32).reshape(OUT_CH, 1),
            "wls": np.asarray(W_ls, np.float32),
            "bls": np.asarray(b_ls, np.float32).reshape(OUT_CH, 1),
            "disp": disp, "disb": disb, "iot": iot, "ident": ident,
            "idx1": pp["idx1"][c], "rel1": pp["rel1"][c],
            "idx2": pp["idx2"][c], "rel2": pp["rel2"][c],
        })
    return in_maps


def kernel(x, edge_index, W1, b1, W_mu, b_mu, W_ls, b_ls):
    from concourse.bass_utils import run_bass_kernel_spmd

    pp = preprocess(np.asarray(edge_index))
    nc = build_program(pp["K1"], pp["K2"], pp["T1"], pp["T2"])
    globals()["_LAST_NC"] = nc
    in_maps = make_in_maps(x, W1, b1, W_mu, b_mu, W_ls, b_ls, pp)
    res = run_bass_kernel_spmd(nc, in_maps, list(range(N_CORES)))
    mu = np.concatenate(
        [res.results[c]["mu_t"][:, :NPC].T for c in range(N_CORES)], axis=0)
    ls = np.concatenate(
        [res.results[c]["ls_t"][:, :NPC].T for c in range(N_CORES)], axis=0)
    return (mu.astype(np.float32), ls.astype(np.float32))


# revision 17
# speedup vs baseline: 1.3810x; 1.3810x over previous
"""GCN encoder (2-layer GCNConv: relu(GCN(x)) -> mu, logstd) on 8 TRN2 NeuronCores.

Strategy (per sharding hint): shard nodes/edges by destination across the 8
cores. Per core:
  Phase A: full h = (x @ W1) * dis  computed redundantly (row-major, DRAM).
  Phase B: edge aggregation for own dst-shard: indexed dma_gather of
           hs[src] rows (f32, 512B descriptors) + one-hot Sel matmul
           segment-reduction on the PE into PSUM (agg^T, channel-major).
  Phase C: h2s = dis * relu(dis*agg1 + b1): own shard, transposed to
           row-major via TensorE.
  Phase D: AllGather h2s shards -> full gather source for layer 2.
  Phase E: layer-2 aggregation (same edge structure, new gather source).
  Phase F: mu^T = W_mu^T @ (dis*agg2) + b_mu, same for logstd; DMA out.
Self-loops are appended to the edge list on the host (weight dis[n]^2 ==
standard edge norm), so no separate self-loop term is needed.
Host does index-only preprocessing: degree counting, sorting edges by
(dst-shard, src-half, dst-group), padding to a core-uniform tile structure.
"""

import os
import sys

sys.path.insert(0, "/opt/trn_rl_repo")

import numpy as np

N_NODES = 50000
N_EDGES = 1_600_000
IN_CH, HID_CH, OUT_CH = 256, 128, 64
N_CORES = 8
NPC = 6250              # real nodes per core
NPC_PAD = 6272          # padded (x64) nodes per core
W_SEL = 64              # Sel width = nodes per PSUM group
GROUPS = NPC_PAD // W_SEL  # 98
TILE_E = 128            # edges per matmul tile
CALL_TILES = 32         # tiles per dma_gather call (4096 idxs)
ROWS = N_NODES + 48     # x rows padded to x128 (50048)
# L1 gather-source layout: nodes 0..25087 at rows 0.., 64 zero rows,
# nodes 25088..50047 at rows 25152.., 64 zero rows. Total 50176.
SPLIT1 = 25088
HS_ROWS = 50176
# L2 source: AllGather output, shard c at rows c*6272, cols 6250.. zeroed.
SPLIT2_NODE = 25000     # nodes < 25000 live in rows < 25088


def _ceil(a, b):
    return -(-a // b)


def preprocess(edge_index):
    """Index-only host preprocessing. Returns per-core device arrays and the
    compile-time tile structure (core-uniform)."""
    src = edge_index[0].astype(np.int64)
    dst = edge_index[1].astype(np.int64)
    deg = 1.0 + np.bincount(dst, minlength=N_NODES).astype(np.float64)
    dis = (1.0 / np.sqrt(deg)).astype(np.float32)

    sl = np.arange(N_NODES, dtype=np.int64)
    src_a = np.concatenate([src, sl])
    dst_a = np.concatenate([dst, sl])

    core = dst_a // NPC
    dstloc = dst_a % NPC
    grp = dstloc // W_SEL
    rel = (dstloc - grp * W_SEL).astype(np.float32)

    # L1 gather rows (node-major with zero-row gaps); L2 rows (shard-major).
    row1 = src_a + 64 * (src_a >= SPLIT1)
    half1 = (src_a >= SPLIT1).astype(np.int64)
    loc1 = row1 - half1 * (SPLIT1 + 64)
    row2 = (src_a // NPC) * NPC_PAD + (src_a % NPC)
    half2 = (src_a >= SPLIT2_NODE).astype(np.int64)
    loc2 = row2 - half2 * SPLIT1
    DUMMY1 = (SPLIT1, 24960)       # zero rows, local idx per half
    DUMMY2 = (NPC, NPC)            # shard-0 pad row 6250 / shard-4 pad row

    # Counts per (core, layer-half, group) -> uniform K across cores.
    def structure(half):
        key = (core * 2 + half) * GROUPS + grp
        cnt = np.bincount(key, minlength=N_CORES * 2 * GROUPS).reshape(
            N_CORES, 2, GROUPS)
        K = _ceil(cnt.max(axis=0), TILE_E)  # [2, GROUPS]
        return cnt, K

    cnt1, K1 = structure(half1)
    cnt2, K2 = structure(half2)
    # Round each half's tile total up to CALL_TILES by bumping last group.
    for K in (K1, K2):
        for h in range(2):
            tot = int(K[h].sum())
            K[h, GROUPS - 1] += _ceil(tot, CALL_TILES) * CALL_TILES - tot

    def build_layer(halfl, locl, K, dummy):
        T = [int(K[h].sum()) for h in range(2)]
        idx_all = np.empty((N_CORES, 128, (T[0] + T[1]) * 8), np.int16)
        rel_all = np.zeros((N_CORES, 128, T[0] + T[1]), np.float32)
        # group start offsets (in edges) within each half stream
        off = [np.concatenate([[0], np.cumsum(K[h]) * TILE_E]) for h in (0, 1)]
        order = np.lexsort((grp, halfl, core))
        s_core, s_half, s_grp = core[order], halfl[order], grp[order]
        s_loc, s_rel = locl[order], rel[order]
        # boundaries per (core, half, group)
        key = (s_core * 2 + s_half) * GROUPS + s_grp
        starts = np.searchsorted(key, np.arange(N_CORES * 2 * GROUPS + 1))
        for c in range(N_CORES):
            flat_idx = np.empty((T[0] + T[1]) * TILE_E, np.int64)
            flat_rel = np.zeros((T[0] + T[1]) * TILE_E, np.float32)
            for h in range(2):
                base = 0 if h == 0 else T[0] * TILE_E
                flat_idx[base:base + T[h] * TILE_E] = dummy[h]
                for g in range(GROUPS):
                    k = (c * 2 + h) * GROUPS + g
                    a, b = starts[k], starts[k + 1]
                    n = b - a
                    o = base + int(off[h][g])
                    flat_idx[o:o + n] = s_loc[a:b]
                    flat_rel[o:o + n] = s_rel[a:b]
            ia = flat_idx.reshape(-1, 16).T.astype(np.int16)  # [16, T*8]
            idx_all[c] = np.tile(ia, (8, 1))
            rel_all[c] = flat_rel.reshape(-1, 128).T
        return T, idx_all, rel_all

    T1, idx1, rel1 = build_layer(half1, loc1, K1, DUMMY1)
    T2, idx2, rel2 = build_layer(half2, loc2, K2, DUMMY2)
    return dict(dis=dis, K1=K1, K2=K2, T1=T1, T2=T2,
                idx1=idx1, rel1=rel1, idx2=idx2, rel2=rel2)


def build_program(K1, K2, T1, T2, phases="ABCDEF"):
    import concourse.bass as bass
    import concourse.bacc as bacc
    import concourse.mybir as mybir
    import concourse.tile as tile

    DT = mybir.dt
    F32 = DT.float32
    nc = bacc.Bacc(None, target_bir_lowering=False, debug=False)

    xT = nc.declare_dram_parameter("xT", [IN_CH, ROWS], F32, isOutput=False)
    w1 = nc.declare_dram_parameter("w1", [IN_CH, HID_CH], F32, isOutput=False)
    b1 = nc.declare_dram_parameter("b1", [HID_CH, 1], F32, isOutput=False)
    wmu = nc.declare_dram_parameter("wmu", [HID_CH, OUT_CH], F32, isOutput=False)
    bmu = nc.declare_dram_parameter("bmu", [OUT_CH, 1], F32, isOutput=False)
    wls = nc.declare_dram_parameter("wls", [HID_CH, OUT_CH], F32, isOutput=False)
    bls = nc.declare_dram_parameter("bls", [OUT_CH, 1], F32, isOutput=False)
    disp = nc.declare_dram_parameter("disp", [128, ROWS // 128], F32, isOutput=False)
    disb = nc.declare_dram_parameter("disb", [128, NPC_PAD], F32, isOutput=False)
    iot = nc.declare_dram_parameter("iot", [128, W_SEL], F32, isOutput=False)
    ident = nc.declare_dram_parameter("ident", [128, 128], F32, isOutput=False)
    idx1e = nc.declare_dram_parameter("idx1", [128, (T1[0] + T1[1]) * 8], DT.int16, isOutput=False)
    rel1e = nc.declare_dram_parameter("rel1", [128, T1[0] + T1[1]], F32, isOutput=False)
    idx2e = nc.declare_dram_parameter("idx2", [128, (T2[0] + T2[1]) * 8], DT.int16, isOutput=False)
    rel2e = nc.declare_dram_parameter("rel2", [128, T2[0] + T2[1]], F32, isOutput=False)
    mu_t = nc.declare_dram_parameter("mu_t", [OUT_CH, NPC_PAD], F32, isOutput=True)
    ls_t = nc.declare_dram_parameter("ls_t", [OUT_CH, NPC_PAD], F32, isOutput=True)

    hs1 = nc.dram_tensor("hs1", [HS_ROWS, HID_CH], F32)

    with tile.TileContext(nc, num_cores=N_CORES) as tc:
        with (
            tc.tile_pool(name="const", bufs=1) as cpool,
            tc.tile_pool(name="xa", bufs=2) as xpool,
            tc.tile_pool(name="hst", bufs=2) as hstpool,
            tc.tile_pool(name="pa", bufs=2, space="PSUM") as papool,
            tc.tile_pool(name="ha", bufs=3) as hapool,
            tc.tile_pool(name="gb", bufs=2) as gpool,
            tc.tile_pool(name="ib", bufs=3) as ipool,
            tc.tile_pool(name="sel", bufs=6) as spool,
            tc.tile_pool(name="pg", bufs=2, space="PSUM") as pgpool,
            tc.tile_pool(name="agg", bufs=1) as apool,
            tc.tile_pool(name="dram", bufs=1, space="DRAM") as dpool,
            tc.tile_pool(name="pt", bufs=2, space="PSUM") as ptpool,
            tc.tile_pool(name="po", bufs=2, space="PSUM") as popool,
            tc.tile_pool(name="outp", bufs=3) as opool,
        ):
            # ---- constants ----
            w1_sb = cpool.tile([128, 2, HID_CH], F32)  # two k-tiles
            nc.sync.dma_start(w1_sb[:, 0, :], w1[0:128, :])
            nc.sync.dma_start(w1_sb[:, 1, :], w1[128:256, :])
            b1_sb = cpool.tile([HID_CH, 1], F32)
            nc.sync.dma_start(b1_sb[:], b1[:])
            wmu_sb = cpool.tile([HID_CH, OUT_CH], F32)
            nc.sync.dma_start(wmu_sb[:], wmu[:])
            bmu_sb = cpool.tile([OUT_CH, 1], F32)
            nc.sync.dma_start(bmu_sb[:], bmu[:])
            wls_sb = cpool.tile([HID_CH, OUT_CH], F32)
            nc.sync.dma_start(wls_sb[:], wls[:])
            bls_sb = cpool.tile([OUT_CH, 1], F32)
            nc.sync.dma_start(bls_sb[:], bls[:])
            disp_sb = cpool.tile([128, ROWS // 128], F32)
            nc.sync.dma_start(disp_sb[:], disp[:])
            disb_sb = cpool.tile([128, NPC_PAD], F32)
            nc.sync.dma_start(disb_sb[:], disb[:])
            io_sb = cpool.tile([128, W_SEL], F32)
            nc.sync.dma_start(io_sb[:], iot[:])
            id_sb = cpool.tile([128, 128], F32)
            nc.sync.dma_start(id_sb[:], ident[:])
            zero_sb = cpool.tile([128, HID_CH], F32)
            nc.vector.memset(zero_sb[:], 0.0)
            # zero rows in hs1
            nc.sync.dma_start(hs1[SPLIT1:SPLIT1 + 64, :], zero_sb[0:64, :])
            nc.sync.dma_start(hs1[HS_ROWS - 64:HS_ROWS, :], zero_sb[0:64, :])

            # ---- Phase A: hs1 = (x @ W1) * dis, row-major, all rows ----
            # Batched: 16 row-tiles (2048 rows of x^T) per DMA group. 391
            # row-tiles; SPLIT1 = 196 tiles is NOT a multiple of 16, so
            # batch hs-writes in runs that do not cross the zero-row gap.
            RT_TOT = ROWS // 128 if "A" in phases else 2
            GRP_RT = 8
            rt = 0
            while rt < RT_TOT:
                n_rt = min(GRP_RT, RT_TOT - rt,
                           (196 - rt) if rt < 196 else RT_TOT)
                cols = slice(rt * 128, (rt + n_rt) * 128)
                xk = xpool.tile([128, 2, GRP_RT * 128], F32, tag="xk")
                nc.sync.dma_start(xk[:, 0, 0:n_rt * 128], xT[0:128, cols])
                nc.sync.dma_start(xk[:, 1, 0:n_rt * 128], xT[128:256, cols])
                hstage = hstpool.tile([128, GRP_RT, HID_CH], F32, tag="hstage")
                for j in range(n_rt):
                    ps = papool.tile([128, HID_CH], F32, tag="psA")
                    nc.tensor.matmul(ps[:], xk[:, 0, j * 128:(j + 1) * 128],
                                     w1_sb[:, 0, :], start=True, stop=False)
                    nc.tensor.matmul(ps[:], xk[:, 1, j * 128:(j + 1) * 128],
                                     w1_sb[:, 1, :], start=False, stop=True)
                    nc.scalar.activation(
                        hstage[:, j, :], ps[:],
                        mybir.ActivationFunctionType.Identity,
                        scale=disp_sb[:, rt + j:rt + j + 1])
                lo = rt * 128 + (64 if rt * 128 >= SPLIT1 else 0)
                dst = hs1[lo:lo + n_rt * 128, :].rearrange(
                    "(j p) c -> p j c", p=128)
                nc.sync.dma_start(dst, hstage[:, 0:n_rt, :])
                rt += n_rt

            # ---- aggregation phase (shared for both layers) ----
            def aggregate(K, T, idx_ext, rel_ext, srcA, srcB, aggT):
                rel_sb = cpool.tile([128, T[0] + T[1]], F32,
                                    tag=f"rel{idx_ext.name}")
                nc.sync.dma_start(rel_sb[:], rel_ext[:])
                for h, src_ap in ((0, srcA), (1, srcB)):
                    tbase = 0 if h == 0 else T[0]
                    n_calls = T[h] // CALL_TILES
                    # tile t (within half) -> (group, k) schedule
                    sched = []
                    for g in range(GROUPS):
                        for k in range(int(K[h][g])):
                            sched.append((g, k, k == 0, k == int(K[h][g]) - 1))
                    assert len(sched) == T[h]
                    ps = None
                    for call in range(n_calls):
                        ixt = ipool.tile([128, CALL_TILES * 8], DT.int16, tag="ixt")
                        c0 = (tbase + call * CALL_TILES) * 8
                        nc.sync.dma_start(ixt[:], idx_ext[:, c0:c0 + CALL_TILES * 8])
                        G = gpool.tile([128, CALL_TILES, HID_CH], F32, tag="G")
                        nc.gpsimd.dma_gather(
                            G[:], src_ap, ixt[:],
                            CALL_TILES * TILE_E, CALL_TILES * TILE_E, HID_CH,
                            single_packet=False)
                        for tl in range(CALL_TILES):
                            t = call * CALL_TILES + tl
                            g, k, first, last = sched[t]
                            sel = spool.tile([128, W_SEL], F32, tag="sel")
                            nc.vector.tensor_scalar(
                                out=sel[:], in0=io_sb[:],
                                scalar1=rel_sb[:, tbase + t:tbase + t + 1],
                                scalar2=None, op0=mybir.AluOpType.is_equal)
                            if first:
                                ps = pgpool.tile([128, W_SEL], F32, tag="psG")
                            nc.tensor.matmul(ps[:], G[:, tl, :], sel[:],
                                             start=first, stop=last)
                            if last:
                                sl = aggT[:, g * W_SEL:(g + 1) * W_SEL]
                                if h == 0:
                                    nc.vector.tensor_copy(sl, ps[:])
                                else:
                                    nc.vector.tensor_tensor(
                                        out=sl, in0=sl, in1=ps[:],
                                        op=mybir.AluOpType.add)
                    # groups never opened in half A (K=0): zero them
                    if h == 0:
                        for g in range(GROUPS):
                            if int(K[0][g]) == 0:
                                nc.vector.memset(
                                    aggT[:, g * W_SEL:(g + 1) * W_SEL], 0.0)

            # ---- Phase B: layer-1 aggregation ----
            agg1 = apool.tile([128, NPC_PAD], F32)
            if "B" in phases:
                aggregate(K1, T1, idx1e, rel1e,
                          hs1[0:SPLIT1 + 64, :], hs1[SPLIT1 + 64:HS_ROWS, :],
                          agg1)
            else:
                nc.vector.memset(agg1[:], 0.0)

            # ---- Phase C: h2s = dis * relu(dis*agg1 + b1), transpose ----
            h2s = agg1  # in-place to save SBUF
            nc.vector.tensor_tensor(out=h2s[:], in0=agg1[:], in1=disb_sb[:],
                                    op=mybir.AluOpType.mult)
            nc.scalar.activation(h2s[:], h2s[:],
                                 mybir.ActivationFunctionType.Relu,
                                 bias=b1_sb[:])
            nc.vector.tensor_tensor(out=h2s[:], in0=h2s[:], in1=disb_sb[:],
                                    op=mybir.AluOpType.mult)
            ag_in = dpool.tile([NPC_PAD, HID_CH], F32)
            ag_out = dpool.tile([N_CORES * NPC_PAD, HID_CH], F32,
                                addr_space="Shared")
            for j in range(NPC_PAD // 128 if "C" in phases else 2):
                pt = ptpool.tile([128, 128], F32, tag="pt")
                nc.tensor.transpose(pt[:], h2s[:, j * 128:(j + 1) * 128], id_sb[:])
                rowt = hapool.tile([128, 128], F32, tag="rowt")
                nc.vector.tensor_copy(rowt[:], pt[:])
                nc.sync.dma_start(ag_in[j * 128:(j + 1) * 128, :], rowt[:])

            # ---- Phase D: AllGather ----
            if "D" in phases:
                nc.gpsimd.collective_compute(
                    "AllGather", mybir.AluOpType.bypass,
                    replica_groups=[list(range(N_CORES))],
                    ins=[ag_in.opt()], outs=[ag_out.opt()])
            else:
                nc.sync.dma_start(ag_out[0:NPC_PAD, :], ag_in[:])

            # ---- Phase E: layer-2 aggregation ----
            agg2 = apool.tile([128, NPC_PAD], F32)
            if "E" in phases:
                aggregate(K2, T2, idx2e, rel2e,
                          ag_out[0:SPLIT1, :],
                          ag_out[SPLIT1:N_CORES * NPC_PAD, :], agg2)
            else:
                nc.vector.memset(agg2[:], 0.0)

            # ---- Phase F: outputs ----
            nc.vector.tensor_tensor(out=agg2[:], in0=agg2[:], in1=disb_sb[:],
                                    op=mybir.AluOpType.mult)
            for (w_sb, b_sb, out_ext) in ((wmu_sb, bmu_sb, mu_t),
                                          (wls_sb, bls_sb, ls_t)):
                for j in range(_ceil(NPC_PAD, 512)):
                    n0 = j * 512
                    n1 = min(NPC_PAD, n0 + 512)
                    po = popool.tile([OUT_CH, 512], F32, tag="po")
                    nc.tensor.matmul(po[:, 0:n1 - n0], w_sb[:],
                                     agg2[:, n0:n1], start=True, stop=True)
                    ot = opool.tile([OUT_CH, 512], F32, tag="ot")
                    nc.vector.tensor_scalar(
                        out=ot[:, 0:n1 - n0], in0=po[:, 0:n1 - n0],
                        scalar1=b_sb[:], scalar2=None,
                        op0=mybir.AluOpType.add)
                    nc.sync.dma_start(out_ext[:, n0:n1], ot[:, 0:n1 - n0])
    nc.compile()
    return nc


def make_in_maps(x, W1, b1, W_mu, b_mu, W_ls, b_ls, pp):
    dis = pp["dis"]
    dis_pad = np.zeros(ROWS, np.float32)
    dis_pad[:N_NODES] = dis
    xT_full = np.zeros((IN_CH, ROWS), np.float32)
    xT_full[:, :N_NODES] = np.asarray(x, np.float32).T
    disp = dis_pad.reshape(ROWS // 128, 128).T.copy()
    iot = np.tile(np.arange(W_SEL, dtype=np.float32)[None, :], (128, 1))
    ident = np.eye(128, dtype=np.float32)
    in_maps = []
    for c in range(N_CORES):
        disb = np.zeros((128, NPC_PAD), np.float32)
        disb[:, :NPC] = np.tile(dis[c * NPC:(c + 1) * NPC][None, :], (128, 1))
        in_maps.append({
            "xT": xT_full, "w1": np.asarray(W1, np.float32),
            "b1": np.asarray(b1, np.float32).reshape(HID_CH, 1),
            "wmu": np.asarray(W_mu, np.float32),
            "bmu": np.asarray(b_mu, np.float32).reshape(OUT_CH, 1),
            "wls": np.asarray(W_ls, np.float32),
            "bls": np.asarray(b_ls, np.float32).reshape(OUT_CH, 1),
            "disp": disp, "disb": disb, "iot": iot, "ident": ident,
            "idx1": pp["idx1"][c], "rel1": pp["rel1"][c],
            "idx2": pp["idx2"][c], "rel2": pp["rel2"][c],
        })
    return in_maps


def kernel(x, edge_index, W1, b1, W_mu, b_mu, W_ls, b_ls):
    from concourse.bass_utils import run_bass_kernel_spmd

    pp = preprocess(np.asarray(edge_index))
    nc = build_program(pp["K1"], pp["K2"], pp["T1"], pp["T2"])
    globals()["_LAST_NC"] = nc
    in_maps = make_in_maps(x, W1, b1, W_mu, b_mu, W_ls, b_ls, pp)
    res = run_bass_kernel_spmd(nc, in_maps, list(range(N_CORES)))
    mu = np.concatenate(
        [res.results[c]["mu_t"][:, :NPC].T for c in range(N_CORES)], axis=0)
    ls = np.concatenate(
        [res.results[c]["ls_t"][:, :NPC].T for c in range(N_CORES)], axis=0)
    return (mu.astype(np.float32), ls.astype(np.float32))
